# revision 96
# baseline (speedup 1.0000x reference)
"""Trainium2 Bass kernel for nn_AbstractFullyConnected (DeepPoly abstract
interpretation of a 5-layer MLP, FC = [784, 2048, 2048, 2048, 2048, 10]).

Strategy (8 NeuronCores, tensor-parallel, fp8e4m3 DoubleRow):
  * Each layer-i bound computation is a back-substitution chain of GEMMs.
    The chain state is kept TRANSPOSED ("G-form": G[k, r] = M[r, k]) so the
    natural [out, in] weight layout serves directly as the matmul stationary
    operand (out = lhsT.T @ rhs) and no per-step transposes are needed.
  * The chain's output rows (layer-i out dim, 2048) are sharded 256/core; the
    low and high chains are stacked along the free dim (2 x 256 = 512 cols),
    so every chain GEMM is [K=2048] x [M=2048 or 784] x [N=512] per core.
  * All chain GEMMs run fp8e4m3 with perf_mode=DoubleRow: lhsT [128,2,128]
    and rhs [128,2,512] contract two K-planes per matmul at 0.5 cycles/row.
    Weights are host-quantized as W*512, chain states carry a 512x scale
    (all backsub states measure max|.| <= 0.2, so 512x sits comfortably in
    fp8e4m3's [subnormal 0.002, max 240] envelope); PSUM drains fold 1/512
    into the relu coefficients (B-set), so no extra rescale ops exist.
  * W1..W4 are fp8 SBUF residents (13.75 MB total), loaded once on the Pool
    (GpSimd) DMA queue, which also runs the fp8 state-combine ops - the
    Activation/Vector engines keep only the PSUM-reading relu/min scalings.
  * Layer-1 bounds are the exact input box: computed host-side in fp32 and
    shipped as a tiny XB1 input, eliminating chain 1 and one AllGather.
  * After each of layers 2..4 only the bound VECTORS (~3 KB) are
    AllGathered; coefficients (c1/c2 + scaled variants + interleaved
    chain-5 pairs) are computed once per layer on DVE/Pool.
  * Chain 5 (10-wide) is computed REDUNDANTLY on every core from the fp8
    residents - zero collectives - with its tiny per-k-tile relu scalings
    BATCHED into [128,320] ops via stride-0 broadcast coefficient APs.
  * Bias/x accumulation uses "row form": lhsT is a [128,N] slice of the
    scaled state, rhs a [128,1] coefficient column, accumulated into one
    PSUM bank's columns across the whole chain.
"""

import contextlib
import os

import numpy as np

MEAN, STD = 0.1307, 0.3081
N_CORES = 8
R = 256          # chain rows per core (2048 / 8)
NT = 16          # k-tiles for 2048
NT1 = 7          # k-tiles for 784 (padded to 896)
IN1P = 896
GRP = 4          # j-tiles per PSUM group in the chain GEMMs

# fp8e4m3 scaling: weights are stored as W*SW, chain states as SG*M.  All
# backsub state matrices measure max|.| <= 0.2 on the reference data, and
# |W| <= 0.2, so 512x scaling puts everything in ~[3e-5, 102] - comfortably
# inside TRN fp8_e4m3's [0.0156(normal)/0.002(subnormal), 240] envelope.
SW = 512.0       # weight scale (W1/W2/W3 fp8 residents)
SG = 512.0       # state scale (git start states + Gs chain states)
INV_SW = 1.0 / SW
INV_SG = 1.0 / SG
INV_SWSG = 1.0 / (SW * SG)

_CACHE = {}


# ----------------------------------------------------------------------------
# walrus in this container supports only ONE sync-wait per instruction; hoist
# extra waits emitted by the Tile scheduler into standalone single-wait
# EventSemaphore instructions placed just before the owning instruction.
# ----------------------------------------------------------------------------
def _split_multiwaits(nc):
    import concourse.mybir as mybir

    n = 0
    for f in nc.m.functions:
        for b in f.blocks:
            insts = list(b.instructions)
            if not any(
                (i.sync_info is not None and len(i.sync_info.on_wait) > 1)
                for i in insts
            ):
                continue
            new = []
            for i in insts:
                si = i.sync_info
                if si is not None and len(si.on_wait) > 1:
                    waits = list(si.on_wait)
                    for k, w in enumerate(waits[:-1]):
                        ev = mybir.InstEventSemaphore(
                            name=f"{i.name}_hw{k}", ins=[], outs=[]
                        )
                        ev.engine = i.engine
                        ev.sync_info = mybir.SyncInfo(on_wait=[w], on_update=[])
                        new.append(ev)
                        n += 1
                    i.sync_info = mybir.SyncInfo(
                        on_wait=[waits[-1]], on_update=list(si.on_update)
                    )
                new.append(i)
            b.instructions = new
    return n


def build_nc():
    KREP = int(os.environ.get("BASS_KREP", "1"))
    NOCC = bool(int(os.environ.get("BASS_NOCC", "0")))
    import concourse.bass as bass
    import concourse.mybir as mybir
    import concourse.tile as tile

    F32 = mybir.dt.float32
    BF = mybir.dt.bfloat16
    F8 = mybir.dt.float8e4
    AF = mybir.ActivationFunctionType
    ALU = mybir.AluOpType
    PM = mybir.MatmulPerfMode

    nc = bass.Bass("TRN2", target_bir_lowering=False, debug=False,
                   num_devices=N_CORES)

    # ---------------- DRAM I/O ----------------
    def din(name, shape, dt=BF):
        return nc.dram_tensor(name, shape, dt, kind="ExternalInput").ap()

    # all weight tensors arrive HOST-PRE-TRANSPOSED into the exact SBUF
    # layout (partition-major, contiguous >=512B chunks -> full DMA bw)
    W1r = din("W1r", [128, NT1 * NT * 128], F8)
    W2r = din("W2r", [128, NT * NT * 128], F8)
    W3r = din("W3r", [NT, 128, 2048], F8)
    W4r = din("W4r", [NT, 128, 2048], F8)
    GTr = {i: din(f"G{i}Tr", [128, NT * R]) for i in (2, 3, 4)}
    # layer-1 bounds+x, exact box computed on host: [lo(16)|hi(16)|x(16)]
    XB1d = din("XB1", [128, 48], F32)
    W5r = din("W5r", [128, 160])
    # b_l / SG, for the bias-vs-b_l matmuls whose lhsT is the SG-scaled
    # fp8 chain state
    bcsc = {l: din(f"bcsc{l}", [128, 16]) for l in (1, 2, 3, 4)}
    btil = {i: din(f"btil{i}", [128, 16], F32) for i in (1, 2, 3, 4)}
    b5d = din("b5", [10, 1], F32)
    # packed input vectors (host-normalized, bf16):
    #   V3 cols 3t+{0,1,2} = (mid0, mid0, x0); V2 cols 2t+{0,1} = (-half0, half0)
    V3d = din("V3", [128, 3 * NT1])
    V2d = din("V2", [128, 2 * NT1])

    out_d = nc.dram_tensor("out", [3, 10], F32, kind="ExternalOutput").ap()

    # internal DRAM for collectives
    ag_in = {i: nc.dram_tensor(f"ag{i}_in", [2, 384], F32).ap()
             for i in (1, 2, 3, 4)}
    ag_out = {i: nc.dram_tensor(f"ag{i}_out", [16, 384], F32,
                                addr_space="Shared").ap()
              for i in (1, 2, 3, 4)}
    rg = [list(range(N_CORES))]

    with tile.TileContext(nc) as tc, contextlib.ExitStack() as est:
        pool_c = est.enter_context(tc.tile_pool(name="const", bufs=1))
        pool_git = est.enter_context(tc.tile_pool(name="git", bufs=2))

        pool_gs = est.enter_context(tc.tile_pool(name="gs", bufs=3))
        pool_ab = est.enter_context(tc.tile_pool(name="ab", bufs=2))
        pool_misc = est.enter_context(tc.tile_pool(name="misc", bufs=2))
        pool_ps = est.enter_context(tc.tile_pool(name="ps", bufs=7, space="PSUM"))
        pool_bb = est.enter_context(tc.tile_pool(name="bb", bufs=1, space="PSUM"))

        # ---------------- constants / resident weights ----------------
        V3 = pool_c.tile([128, 3 * NT1], BF, tag="V3", name="V3")
        V2 = pool_c.tile([128, 2 * NT1], BF, tag="V2", name="V2")
        nc.sync.dma_start(V3[:, :], V3d[:, :])
        nc.sync.dma_start(V2[:, :], V2d[:, :])
        XB1 = pool_c.tile([128, 48], F32, tag="XB1", name="XB1")
        nc.sync.dma_start(XB1[:, :], XB1d[:, :])

        def mid0(t):
            return V3[:, 3 * t:3 * t + 1]

        def x0v(t):
            return V3[:, 3 * t + 2:3 * t + 3]

        def nhalf0(t):
            return V2[:, 2 * t:2 * t + 1]

        def half0(t):
            return V2[:, 2 * t + 1:2 * t + 2]

        btile = {}
        for i in (1, 2, 3, 4):
            t = pool_c.tile([128, 16], F32, tag=f"btil{i}", name=f"btil{i}")
            nc.scalar.dma_start(t[:, :], btil[i][:, :])
            btile[i] = t
        b5t = pool_c.tile([10, 1], F32, tag="b5t", name="b5t")
        nc.scalar.dma_start(b5t[:, :], b5d[:, :])

        # w5buf's DMA is emitted in the chain-4 prologue
        w5buf = pool_c.tile([128, 16 * 10], BF, tag="w5buf", name="w5buf")

        # chain-2 start state: prefetch before chain 1
        git = {}
        git[2] = pool_git.tile([128, NT * R], BF, tag="git", name="git2")
        # halves on different queues: h0 (needed first by the fb) lands on
        # the short SP queue, h1 behind the small consts on Act
        nc.sync.dma_start(git[2][:, 0:2048], GTr[2][:, 0:2048])
        nc.scalar.dma_start(git[2][:, 2048:4096], GTr[2][:, 2048:4096])
        bcscs = {}
        for l in (1, 2, 3, 4):
            t = pool_c.tile([128, 16], BF, tag=f"bcsc{l}", name=f"bcsc{l}")
            nc.scalar.dma_start(t[:, :], bcsc[l][:, :])
            bcscs[l] = t

        # resident full weights W1 (padded 896x2048) and W2 (2048x2048), bf16,
        # streamed on the Activation HWDGE queue so the latency-critical
        # gather/collective DMAs on the SP queue are never stuck behind them.
        W1res = pool_c.tile([128, NT1 * NT * 128], F8, tag="W1res", name="W1res")
        for j in range(NT1):
            nc.gpsimd.dma_start(W1res[:, j * 2048:(j + 1) * 2048],
                                W1r[:, j * 2048:(j + 1) * 2048])
        # W2/W3/W4 are fp8 RESIDENTS, loaded progressively during earlier
        # chains (W2 first needed in chain 3; W3 in chain 4; W4 in chain 5)
        W2res = pool_c.tile([128, NT * NT * 128], F8, tag="W2res", name="W2res")
        W3res = pool_c.tile([128, NT * NT * 128], F8, tag="W3res", name="W3res")
        W4res = pool_c.tile([128, NT * NT * 128], F8, tag="W4res", name="W4res")

        def load_w2res_tiles(js):
            for j in js:
                nc.gpsimd.dma_start(W2res[:, j * 2048:(j + 1) * 2048],
                                    W2r[:, j * 2048:(j + 1) * 2048])

        def load_res_tiles(res, src, js):
            for j in js:
                nc.gpsimd.dma_start(res[:, j * 2048:(j + 1) * 2048], src[j])

        # per-layer relu coefficient tiles (filled after each layer)
        coef = {}
        for i in (1, 2, 3, 4):
            coef[i] = {}
            for k in ("c1", "c2", "nc2", "c1b", "c2b", "nc2b"):
                coef[i][k] = pool_c.tile([128, 16], F32, tag=f"cf{i}{k}",
                                         name=f"cf{i}{k}")
            for k in ("rhv", "rhv_s", "xr_s"):
                coef[i][k] = pool_c.tile([128, 16], BF, tag=f"cf{i}{k}",
                                         name=f"cf{i}{k}")
            for k in ("cm0", "cn0n", "cmb", "cnbn"):
                coef[i][k] = pool_c.tile([128, 32], BF, tag=f"cf{i}{k}",
                                         name=f"cf{i}{k}")

        # ---------------- helpers ----------------
        class BiasCols:
            """bias accumulation columns in one PSUM bank.
            cols 0,1: low m0/m1 | 2,3: high | 4,5: x  (col = 2*kind + m)"""

            def __init__(self, np_part=128):
                self.t = pool_bb.tile([128, 8], F32, tag="bbm", name="bbm")
                # start=True zeroes the WHOLE PSUM bank on this HW, so emit
                # exactly one start for the bank; later first-touches rely on
                # the bank-wide has_written clear (first write = overwrite).
                self.bank_first = True
                self.np_part = np_part

            def mm(self, col, lhsT, rhs, stop=False):
                nc.tensor.matmul(
                    self.t[0:self.np_part, col:col + 1], lhsT, rhs,
                    start=self.bank_first, stop=stop,
                )
                self.bank_first = False

        def relu_pass(dst, src, scale):
            nc.scalar.activation(dst, src, AF.Relu, scale=scale)

        def do_allgather(in_ap, out_ap, rows_per_rank):
            if NOCC:
                # timing-only stub: copy own shard into its slot
                nc.sync.dma_start(out_ap[0:rows_per_rank], in_ap[0:rows_per_rank])
            else:
                nc.gpsimd.collective_compute(
                    "AllGather", ALU.bypass, replica_groups=rg,
                    ins=[in_ap], outs=[out_ap])

        def gather_layer(i, br):
            """DMA bias cols out, AllGather, read back + add b_i, coeffs."""
            # sh col = m*3 + kind (reordered at copy time so the single
            # out-DMA below is stride-uniform on both sides)
            sh = pool_misc.tile([128, 6], F32, tag="sh", name="sh", bufs=1)
            nc.vector.tensor_copy(
                sh[:, :].rearrange("p (m k) -> p m k", m=2),
                br.t[:, 0:6].rearrange("p (k m) -> p m k", k=3))
            # ag row m = [lo(128) | hi(128) | x(128)]
            nc.sync.dma_start(
                ag_in[i][:, :].rearrange("m (k p) -> p m k", k=3),
                sh[:, :].rearrange("p (m k) -> p m k", m=2),
            )
            do_allgather(ag_in[i][:, :], ag_out[i][:, :], 2)
            # X cols: [lo(16) | hi(16) | x(16)], t = ag row (DMA APs allow
            # at most 3 dims incl. partition, so one DMA per column group)
            X = pool_misc.tile([128, 48], F32, tag="Xg", name="Xg")
            for c, eng in ((0, nc.sync), (1, nc.sync), (2, nc.scalar)):
                eng.dma_start(
                    X[:, 16 * c:16 * (c + 1)],
                    ag_out[i][:, 128 * c:128 * (c + 1)].rearrange("t p -> p t"),
                )
            for c in range(3):
                nc.vector.tensor_tensor(X[:, 16 * c:16 * (c + 1)],
                                        X[:, 16 * c:16 * (c + 1)],
                                        btile[i][:, :], ALU.add)
            compute_coeffs(i, X[:, 32:48], X[:, 0:16], X[:, 16:32])

        _cw = [0]

        def compute_coeffs(i, x, lo, hi):
            C = coef[i]

            def tmp():
                _cw[0] += 1
                return pool_misc.tile([128, 16], F32, tag=f"cw{_cw[0] % 20}",
                                      name=f"cw{_cw[0] % 20}", bufs=1)

            # Short-form coefficients (equivalent to the reference's
            # case-split, verified per sign case):
            #   c1 = relu(hi) / (relu(hi) - min(lo,0) + eps)
            #   c2 = (hi > 0) * (hi + lo >= 0)
            # The eps only matters at hi<=0 where the numerator is already
            # 0, so it never perturbs a live coefficient.  This halves the
            # serial DVE chain on every chain-boundary critical path, and
            # relu(hi) comes from the Activation engine in parallel.
            rh = tmp()
            nc.scalar.activation(rh[:, :], hi[:, :], AF.Relu)
            thp, s, sge = tmp(), tmp(), tmp()
            nc.vector.tensor_scalar(thp[:, :], hi[:, :], 0.0, None, ALU.is_gt)
            nc.vector.tensor_tensor(s[:, :], hi[:, :], lo[:, :], ALU.add)
            nc.vector.tensor_scalar(sge[:, :], s[:, :], 0.0, None, ALU.is_ge)
            nc.vector.tensor_tensor(C["c2"][:, :], sge[:, :], thp[:, :],
                                    ALU.mult)
            nc.vector.tensor_scalar_mul(C["nc2"][:, :], C["c2"][:, :], -1.0)
            mle, dn, r = tmp(), tmp(), tmp()
            nc.vector.tensor_scalar(mle[:, :], lo[:, :], 0.0, 1e-20,
                                    ALU.min, ALU.subtract)
            nc.vector.tensor_tensor(dn[:, :], rh[:, :], mle[:, :],
                                    ALU.subtract)
            nc.vector.reciprocal(r[:, :], dn[:, :])
            nc.vector.tensor_tensor(C["c1"][:, :], rh[:, :], r[:, :],
                                    ALU.mult)
            # B-set: drain coefficients folding out the fp8 weight scale
            nc.vector.tensor_scalar_mul(C["c1b"][:, :], C["c1"][:, :], INV_SW)
            nc.vector.tensor_scalar_mul(C["c2b"][:, :], C["c2"][:, :], INV_SW)
            nc.vector.tensor_scalar_mul(C["nc2b"][:, :], C["c2b"][:, :], -1.0)
            # rhv = relu(-lo) * (hi > 0)   [== ub_int / ub_slope, no division]
            rl = tmp()
            nc.scalar.activation(rl[:, :], lo[:, :], AF.Relu, scale=-1.0)
            nc.vector.tensor_tensor(C["rhv"][:, :], rl[:, :], thp[:, :],
                                    ALU.mult)
            # rhv/SG + relu(x)/SG: partners of the SG-scaled AD / git tiles
            nc.vector.tensor_scalar_mul(C["rhv_s"][:, :], C["rhv"][:, :],
                                        INV_SG)
            nc.scalar.activation(C["xr_s"][:, :], x[:, :], AF.Relu,
                                 scale=INV_SG)
            # interleaved [c_lo | c_hi] pairs for chain-5's batched scaling:
            # cm (max-part): lo->c2, hi->c1 ; cn-neg (min-part, NEGATED so
            # the min factor can come from the Activation engine as
            # Relu(-x) = -min(x,0)): lo->-c1, hi->-c2
            nc1n = pool_misc.tile([128, 16], F32, tag="nc1n", name=f"nc1n{i}",
                                  bufs=1)
            nc.vector.tensor_scalar_mul(nc1n[:, :], C["c1"][:, :], -1.0)
            nc1bn = pool_misc.tile([128, 16], F32, tag="nc1bn",
                                   name=f"nc1bn{i}", bufs=1)
            nc.vector.tensor_scalar_mul(nc1bn[:, :], C["c1b"][:, :], -1.0)
            for nm, lo_src, hi_src in (("cm0", C["c2"], C["c1"]),
                                       ("cn0n", nc1n, C["nc2"]),
                                       ("cmb", C["c2b"], C["c1b"]),
                                       ("cnbn", nc1bn, C["nc2b"])):
                v = C[nm][:, :].rearrange("p (t two) -> p two t", two=2)
                nc.gpsimd.tensor_copy(v[:, 0, :], lo_src[:, :])
                nc.gpsimd.tensor_copy(v[:, 1, :], hi_src[:, :])

        # pending small PE matmuls, drained a couple per k-iteration of the
        # subsequent GEMM loops so they interleave with the big-MM stream
        pending = []

        def drain_pending(nmax):
            npop = min(nmax, len(pending))
            for f in pending[:npop]:
                f()
            del pending[:npop]

        for _rep in range(KREP):
            # ======= layer-1 bounds arrive host-precomputed (exact box) ====
            compute_coeffs(1, XB1[:, 32:48], XB1[:, 0:16], XB1[:, 16:32])

            # ================= chains 2..4 =================
            for i in (2, 3, 4):
                g = git[i]
                if i < 4:
                    # prefetch next chain's start state during this chain
                    git[i + 1] = pool_git.tile([128, NT * R], BF, tag="git",
                                               name=f"git{i + 1}")
                    for hh in range(2):
                        nc.sync.dma_start(
                            git[i + 1][:, hh * 2048:(hh + 1) * 2048],
                            GTr[i + 1][:, hh * 2048:(hh + 1) * 2048])
                # W3/W4 resident loads are spread through this chain's
                # GEMM group loop (below) so the Pool queue is never
                # blocked in front of the first-block combines

                bbx = BiasCols()
                cf = coef[i - 1]

                # ---- first block: scale start state g by coef[i-1] ----
                # g arrives host-pre-scaled by SG, so AD/Gs inherit the SG
                # state scale; fb bias matmuls pair them with rhv_s/xr_s.
                # AD[t]: cols [0:256] = D_signed = c1*min(g,0) (<=0),
                #        cols [256:512] = A = c1*max(g,0)
                # Gs[t]: lo = c2*max(g,0) + D_signed ; hi = A - (-c2*min(g,0))
                Gs = pool_gs.tile([128, NT * 512], F8, tag="gs", name="gs")
                for t in range(NT):
                    gsl = g[:, t * R:(t + 1) * R]
                    AD = pool_ab.tile([128, 512], BF, tag="AD", name="AD",
                                      bufs=6)
                    B = pool_ab.tile([128, R], BF, tag="B", name="B", bufs=3)
                    Cc = pool_ab.tile([128, R], BF, tag="C", name="C", bufs=3)
                    nc.vector.tensor_scalar(Cc[:, :], gsl, 0.0,
                                            cf["c2"][:, t:t + 1],
                                            ALU.max, ALU.mult)
                    nc.vector.tensor_scalar(B[:, :], gsl, 0.0,
                                            cf["nc2"][:, t:t + 1],
                                            ALU.min, ALU.mult)
                    relu_pass(AD[:, 256:512], gsl, cf["c1"][:, t:t + 1])
                    nc.vector.tensor_scalar(AD[:, 0:256], gsl, 0.0,
                                            cf["c1"][:, t:t + 1],
                                            ALU.min, ALU.mult)
                    # fp8 state combines run on the (otherwise idle) Pool
                    # engine - SBUF-only, no PSUM involved
                    nc.gpsimd.tensor_tensor(
                        Gs[:, t * 512:t * 512 + 256],
                        Cc[:, :], AD[:, 0:256], ALU.add)
                    nc.gpsimd.tensor_tensor(
                        Gs[:, t * 512 + 256:t * 512 + 512],
                        AD[:, 256:512], B[:, :], ALU.subtract)

                    def fb_mms(t=t, AD=AD, gsl=gsl, cf=cf):
                        last = t == NT - 1
                        for mi in range(2):
                            sl = slice(mi * 128, (mi + 1) * 128)
                            sl2 = slice(256 + mi * 128, 256 + (mi + 1) * 128)
                            bbx.mm(0 + mi, AD[:, sl], cf["rhv_s"][:, t:t + 1])
                            bbx.mm(2 + mi, AD[:, sl2],
                                   cf["rhv_s"][:, t:t + 1])
                            bbx.mm(4 + mi, gsl[:, sl], cf["xr_s"][:, t:t + 1],
                                   stop=last)
                    pending.append(fb_mms)

                # ---- GEMM steps l = i-1 .. 1 ----
                for li, l in enumerate(range(i - 1, 0, -1)):
                    nj = NT if l > 1 else NT1
                    Gs_next = (pool_gs.tile([128, NT * 512], F8, tag="gs",
                                            name="gs") if l > 1 else None)
                    cfl = coef[l - 1] if l > 1 else None

                    def wsl2(j, kk, l=l):
                        # [128, 2, 128] fp8 stationary: two adjacent k-planes
                        res = {1: W1res, 2: W2res, 3: W3res}[l]
                        w = res[:, (j * NT + 2 * kk) * 128:
                                (j * NT + 2 * kk + 2) * 128]
                        return w.rearrange("p (two m) -> p two m", two=2)

                    # bias vs b_l on the current (scaled) state.  These are
                    # ready as soon as Gs is, so put them IN FRONT of any
                    # leftover closures that still wait on the previous
                    # step's last drains - otherwise those block the PE
                    # stream at the start of this step's GEMM.
                    bl_list = []
                    for t in range(NT):
                        def bl_mms(t=t, Gs=Gs, l=l):
                            for mi in range(2):
                                lo_l = Gs[:, t * 512 + mi * 128:
                                          t * 512 + (mi + 1) * 128]
                                hi_l = Gs[:, t * 512 + 256 + mi * 128:
                                          t * 512 + 256 + (mi + 1) * 128]
                                bbx.mm(0 + mi, lo_l, bcscs[l][:, t:t + 1])
                                bbx.mm(2 + mi, hi_l, bcscs[l][:, t:t + 1])
                        bl_list.append(bl_mms)
                    if li == 0:
                        # first step: fb closures are still pending and the
                        # AD-pool WAR means front-running them deadlocks
                        pending.extend(bl_list)
                    else:
                        pending[:0] = bl_list

                    # first group is only 2 j-tiles: the NEXT step's GEMM
                    # needs Gs_next tiles 0,1, so a small lead group cuts
                    # the per-step pipeline-fill latency.  l=1 uses groups
                    # of 2 throughout so the S/T box drains pace with the
                    # GEMM instead of bunching at the chain end.
                    step = GRP if l > 1 else 2
                    bounds = [0, min(2, nj)]
                    while bounds[-1] < nj:
                        bounds.append(min(bounds[-1] + step, nj))
                    groups = [list(range(a, b))
                              for a, b in zip(bounds, bounds[1:])]
                    # all resident loads complete by chain-3's end: the
                    # gather path couples to the Pool DMA counter, so no
                    # Pool DMA should straddle a chain boundary
                    w2sched = {0: range(0, 8), 1: range(8, 16)}
                    sched = {1: range(0, 5), 2: range(5, 10),
                             3: range(10, 16)}
                    sched4 = {0: range(0, 6), 1: range(6, 11),
                              2: range(11, 16)}
                    for gi, grp in enumerate(groups):
                        if i == 2 and gi in w2sched:
                            load_w2res_tiles(w2sched[gi])
                        if i == 3 and li == 0 and gi in sched:
                            load_res_tiles(W3res, W3r, sched[gi])
                        if i == 3 and li == 1 and gi in sched4:
                            load_res_tiles(W4res, W4r, sched4[gi])
                            if gi == 2:
                                nc.scalar.dma_start(w5buf[:, :], W5r[:, :])
                        pss = {}
                        for j in grp:
                            pss[j] = pool_ps.tile([128, 512], F32, tag="ps",
                                                  name="ps")
                        for kk in range(NT // 2):
                            for j in grp:
                                nc.tensor.matmul(
                                    pss[j][:, :], wsl2(j, kk),
                                    Gs[:, kk * 1024:(kk + 1) * 1024]
                                    .rearrange("p (two n) -> p two n", two=2),
                                    start=(kk == 0), stop=(kk == NT // 2 - 1),
                                    perf_mode=PM.DoubleRow)
                            drain_pending(4)
                        # drains + scaling for this group
                        for j in grp:
                            ps = pss[j]
                            if l > 1:
                                AD = pool_ab.tile([128, 512], BF, tag="AD",
                                                  name="AD", bufs=6)
                                B = pool_ab.tile([128, R], BF, tag="B",
                                                 name="B", bufs=3)
                                Cc = pool_ab.tile([128, R], BF, tag="C",
                                                  name="C", bufs=3)
                                hi_sl, lo_sl = ps[:, 256:512], ps[:, 0:256]
                                relu_pass(AD[:, 256:512], hi_sl,
                                          cfl["c1b"][:, j:j + 1])
                                nc.vector.tensor_scalar(AD[:, 0:256], lo_sl,
                                                        0.0,
                                                        cfl["c1b"][:, j:j + 1],
                                                        ALU.min, ALU.mult)
                                relu_pass(Cc[:, :], lo_sl,
                                          cfl["c2b"][:, j:j + 1])
                                nc.vector.tensor_scalar(B[:, :], hi_sl, 0.0,
                                                        cfl["nc2b"][:, j:j + 1],
                                                        ALU.min, ALU.mult)
                                nc.gpsimd.tensor_tensor(
                                    Gs_next[:, j * 512:j * 512 + 256],
                                    Cc[:, :], AD[:, 0:256], ALU.add)
                                nc.gpsimd.tensor_tensor(
                                    Gs_next[:, j * 512 + 256:j * 512 + 512],
                                    AD[:, 256:512], B[:, :], ALU.subtract)

                                def relu_mms(AD=AD, j=j, cfl=cfl):
                                    for mi in range(2):
                                        sl = slice(mi * 128, (mi + 1) * 128)
                                        sl2 = slice(256 + mi * 128,
                                                    256 + (mi + 1) * 128)
                                        bbx.mm(0 + mi, AD[:, sl],
                                               cfl["rhv_s"][:, j:j + 1])
                                        bbx.mm(2 + mi, AD[:, sl2],
                                               cfl["rhv_s"][:, j:j + 1])
                                pending.append(relu_mms)
                            else:
                                S = pool_ab.tile([128, 512], BF, tag="S",
                                                 name="S", bufs=4)
                                T = pool_ab.tile([128, 512], BF, tag="T",
                                                 name="T", bufs=4)
                                # PSUM carries SW*SG x true values here; S on
                                # DVE so the Abs/Copy pair doesn't serialize
                                # on the Activation engine
                                nc.vector.tensor_scalar_mul(S[:, :], ps[:, :],
                                                            INV_SWSG)
                                nc.scalar.activation(T[:, :], ps[:, :],
                                                     AF.Abs, scale=INV_SWSG)

                                def box_mms(S=S, T=T, j=j,
                                            last=(j == NT1 - 1)):
                                    for mi in range(2):
                                        sl = slice(mi * 128, (mi + 1) * 128)
                                        sl2 = slice(256 + mi * 128,
                                                    256 + (mi + 1) * 128)
                                        bbx.mm(0 + mi, S[:, sl], mid0(j))
                                        bbx.mm(0 + mi, T[:, sl], nhalf0(j),
                                               stop=last)
                                        bbx.mm(2 + mi, S[:, sl2], mid0(j))
                                        bbx.mm(2 + mi, T[:, sl2], half0(j),
                                               stop=last)
                                pending.append(box_mms)
                    if l > 1:
                        Gs = Gs_next
                drain_pending(len(pending))
                gather_layer(i, bbx)

            # ====== chain 5 (10-wide, fp8, computed REDUNDANTLY per core ====
            # every core runs the full chain against the fp8 resident
            # weights: ~0.5% of the chain-2..4 FLOPs, and it removes the 4
            # serial AllGather round-trips of the old sharded formulation.
            # All relu-scalings are BATCHED [128,320] ops (max/min split,
            # then two broadcast-coef multiplies) - per-tile op chains would
            # be semaphore-latency bound at this width.
            cf4 = coef[4]

            def ap_add0(v, pos, n):
                # insert a stride-0 (broadcast) dim of size n at position pos
                a = [list(d) for d in v.ap]
                a.insert(pos, [0, n])
                return bass.AP(v.tensor, v.offset, a)

            def v4(t, w=320):   # [128,w] tile as [128, w/20, 2, 10]
                return t[:, 0:w].rearrange("p (t two n) -> p t two n",
                                           two=2, n=10)

            def vx(t):          # [128,160] -> [128,16,{bc 2},10]
                return ap_add0(t[:, :].rearrange("p (t n) -> p t n", n=10),
                               2, 2)

            def vc(t):          # [128,32] -> [128,16,2,{bc 10}]
                return ap_add0(t[:, :].rearrange("p (t two) -> p t two",
                                                 two=2), 3, 10)

            bb5 = BiasCols(np_part=10)
            # ---- first block: lo == hi == W5^T (SG-scaled), batched ----
            mx0 = pool_misc.tile([128, 160], BF, tag="mxf", name="mx5")
            mn0 = pool_misc.tile([128, 160], BF, tag="mnf", name="mn5")
            nc.vector.tensor_scalar(mx0[:, :], w5buf[:, :], 0.0, None,
                                    ALU.max)
            nc.scalar.activation(mn0[:, :], w5buf[:, :], AF.Relu,
                                 scale=-1.0)   # = -min(w,0)
            t1f = pool_misc.tile([128, 320], BF, tag="t15", name="t15f")
            t2f = pool_misc.tile([128, 320], BF, tag="t25", name="t25f")
            G5s = pool_c.tile([128, 16 * 20], F8, tag="g5s", name="g5s")
            nc.vector.tensor_tensor(v4(t1f), vx(mx0), vc(cf4["cm0"]),
                                    ALU.mult)
            nc.gpsimd.tensor_tensor(v4(t2f), vx(mn0), vc(cf4["cn0n"]),
                                    ALU.mult)
            nc.gpsimd.tensor_tensor(G5s[:, :], t1f[:, :], t2f[:, :], ALU.add)
            for t in range(NT):
                # AD parts live inside t1/t2: lo-bias = c1*min (t2 lo cols),
                # hi-bias = c1*max (t1 hi cols)
                bb5.mm(0, t2f[:, t * 20:t * 20 + 10],
                       cf4["rhv_s"][:, t:t + 1])
                bb5.mm(2, t1f[:, t * 20 + 10:t * 20 + 20],
                       cf4["rhv_s"][:, t:t + 1])
                bb5.mm(4, w5buf[:, t * 10:(t + 1) * 10],
                       cf4["xr_s"][:, t:t + 1], stop=(t == NT - 1))

            def w5sl(res, j, kk):
                w = res[:, (j * NT + 2 * kk) * 128:
                        (j * NT + 2 * kk + 2) * 128]
                return w.rearrange("p (two m) -> p two m", two=2)

            for l in (4, 3, 2, 1):
                for t in range(NT):
                    bb5.mm(0, G5s[:, t * 20:t * 20 + 10],
                           bcscs[l][:, t:t + 1])
                    bb5.mm(2, G5s[:, t * 20 + 10:t * 20 + 20],
                           bcscs[l][:, t:t + 1])
                res = {4: W4res, 3: W3res, 2: W2res, 1: W1res}[l]
                njl = NT if l > 1 else NT1
                ps5 = pool_ps.tile([128, 512], F32, tag="ps", name=f"ps5_{l}")
                first = True
                for j in range(njl):
                    for kk in range(NT // 2):
                        nc.tensor.matmul(
                            ps5[:, j * 20:(j + 1) * 20], w5sl(res, j, kk),
                            G5s[:, 2 * kk * 20:(2 * kk + 2) * 20]
                            .rearrange("p (two n) -> p two n", two=2),
                            start=first, stop=(kk == NT // 2 - 1),
                            perf_mode=PM.DoubleRow)
                        first = False
                if l > 1:
                    cfl = coef[l - 1]
                    mx = pool_misc.tile([128, 320], BF, tag="mx5",
                                        name=f"mx{l}")
                    mn = pool_misc.tile([128, 320], BF, tag="mn5",
                                        name=f"mn{l}")
                    nc.vector.tensor_scalar(mx[:, :], ps5[:, 0:320], 0.0,
                                            None, ALU.max)
                    nc.scalar.activation(mn[:, :], ps5[:, 0:320], AF.Relu,
                                         scale=-1.0)   # = -min(ps,0)
                    t1 = pool_misc.tile([128, 320], BF, tag="t15",
                                        name=f"t1{l}")
                    t2 = pool_misc.tile([128, 320], BF, tag="t25",
                                        name=f"t2{l}")
                    nc.vector.tensor_tensor(v4(t1), v4(mx), vc(cfl["cmb"]),
                                            ALU.mult)
                    nc.gpsimd.tensor_tensor(v4(t2), v4(mn), vc(cfl["cnbn"]),
                                            ALU.mult)
                    G5n = pool_c.tile([128, 16 * 20], F8, tag="g5n",
                                      name=f"g5n{l}")
                    nc.gpsimd.tensor_tensor(G5n[:, :], t1[:, :], t2[:, :],
                                            ALU.add)
                    for t in range(NT):
                        bb5.mm(0, t2[:, t * 20:t * 20 + 10],
                               cfl["rhv_s"][:, t:t + 1])
                        bb5.mm(2, t1[:, t * 20 + 10:t * 20 + 20],
                               cfl["rhv_s"][:, t:t + 1])
                    G5s = G5n
                else:
                    S5 = pool_misc.tile([128, 140], BF, tag="s5f", name="S5f")
                    T5 = pool_misc.tile([128, 140], BF, tag="t5f", name="T5f")
                    nc.vector.tensor_scalar_mul(S5[:, :], ps5[:, 0:140],
                                                INV_SWSG)
                    nc.scalar.activation(T5[:, :], ps5[:, 0:140], AF.Abs,
                                         scale=INV_SWSG)
                    for t in range(NT1):
                        last = t == NT1 - 1
                        bb5.mm(0, S5[:, t * 20:t * 20 + 10], mid0(t))
                        bb5.mm(0, T5[:, t * 20:t * 20 + 10], nhalf0(t),
                               stop=last)
                        bb5.mm(2, S5[:, t * 20 + 10:t * 20 + 20], mid0(t))
                        bb5.mm(2, T5[:, t * 20 + 10:t * 20 + 20], half0(t),
                               stop=last)

            # final outputs: out[0]=x5, out[1]=low5, out[2]=high5
            fin = pool_misc.tile([10, 3], F32, tag="fin", name="fin")
            nc.vector.tensor_tensor(fin[:, 0:1], bb5.t[0:10, 4:5], b5t[:, :],
                                    ALU.add)
            nc.vector.tensor_tensor(fin[:, 1:2], bb5.t[0:10, 0:1], b5t[:, :],
                                    ALU.add)
            nc.vector.tensor_tensor(fin[:, 2:3], bb5.t[0:10, 2:3], b5t[:, :],
                                    ALU.add)
            nc.sync.dma_start(out_d.rearrange("k p -> p k"), fin[:, :])

    _split_multiwaits(nc)
    return nc


def make_in_maps(x, low, high, Ws, bs):
    """Host-side shard/layout prep. Ws/bs: dicts 1..5."""
    import ml_dtypes

    BFnp = ml_dtypes.bfloat16
    F8np = ml_dtypes.float8_e4m3

    def to_f8(a):
        s = np.asarray(a, np.float32) * SW
        assert np.abs(s).max() < 235.0, (
            f"fp8 weight scale overflow: max {np.abs(s).max()}")
        return s.astype(F8np)

    def pad_vec(v):
        p = np.zeros(IN1P, np.float32)
        p[:784] = ((np.asarray(v).reshape(-1) - MEAN) / STD)
        return p

    xn, ln, hn = pad_vec(x), pad_vec(low), pad_vec(high)
    mid = 0.5 * (ln + hn)
    half = 0.5 * (hn - ln)

    def tmaj(v):
        return np.ascontiguousarray(v.reshape(16, 128).T.astype(np.float32))
    V3 = np.zeros((128, 3 * NT1), np.float32)
    V2 = np.zeros((128, 2 * NT1), np.float32)
    for t in range(NT1):
        seg = slice(t * 128, (t + 1) * 128)
        V3[:, 3 * t + 0] = mid[seg]
        V3[:, 3 * t + 1] = mid[seg]
        V3[:, 3 * t + 2] = xn[seg]
        V2[:, 2 * t + 0] = -half[seg]
        V2[:, 2 * t + 1] = half[seg]

    W1p = np.zeros((2048, IN1P), np.float32)
    W1p[:, :784] = Ws[1]

    def wres(W):
        # Wres[p, (j k c)] = W[k*128+p, j*128+c]
        nj = W.shape[1] // 128
        return np.ascontiguousarray(
            W.reshape(16, 128, nj, 128).transpose(1, 2, 0, 3)
            .reshape(128, nj * 16 * 128)).astype(BFnp)

    def wstream(W):
        # Wst[j, p, (k c)] = W[k*128+p, j*128+c]
        nj = W.shape[1] // 128
        return np.ascontiguousarray(
            W.reshape(16, 128, nj, 128).transpose(2, 1, 0, 3)
            .reshape(nj, 128, 16 * 128)).astype(BFnp)

    def gform(G):
        # g[p, (t c)] = G[t*128+p, c]   (G = [K, r])
        nt = G.shape[0] // 128
        return np.ascontiguousarray(
            G.reshape(nt, 128, G.shape[1]).transpose(1, 0, 2)
            .reshape(128, -1)).astype(BFnp)

    # exact layer-1 box bounds + forward x, computed host-side in fp32
    W1f = W1p.astype(np.float32)
    lo1 = W1f @ mid - np.abs(W1f) @ half + bs[1]
    hi1 = W1f @ mid + np.abs(W1f) @ half + bs[1]
    x1 = W1f @ xn + bs[1]
    XB1 = np.concatenate([tmaj(lo1), tmaj(hi1), tmaj(x1)], axis=1)

    common = {"XB1": XB1,
              "V3": V3.astype(BFnp), "V2": V2.astype(BFnp),
              "W5r": gform(np.ascontiguousarray(Ws[5].T) * SG),
              "b5": np.ascontiguousarray(bs[5].reshape(10, 1)),
              "W1r": to_f8(wres(W1p).astype(np.float32)),
              "W2r": to_f8(wres(Ws[2]).astype(np.float32)),
              "W3r": to_f8(wstream(Ws[3]).astype(np.float32)),
              "W4r": to_f8(wstream(Ws[4]).astype(np.float32))}
    for l in (1, 2, 3, 4):
        common[f"bc{l}"] = np.ascontiguousarray(
            bs[l].reshape(16, 128).T).astype(BFnp)
        common[f"bcsc{l}"] = np.ascontiguousarray(
            bs[l].reshape(16, 128).T / SG).astype(BFnp)
        common[f"btil{l}"] = np.ascontiguousarray(
            bs[l].reshape(16, 128).T.astype(np.float32))

    maps = []
    for d in range(N_CORES):
        m = dict(common)
        sh = slice(256 * d, 256 * (d + 1))
        for i, W in ((2, Ws[2]), (3, Ws[3]), (4, Ws[4])):
            m[f"G{i}Tr"] = gform(
                np.ascontiguousarray(W[sh, :].T) * SG)
        maps.append(m)
    return maps


def _prep_inputs(inputs):
    Ws = {i: np.asarray(inputs[f"W{i}"], np.float32) for i in range(1, 6)}
    bs = {i: np.asarray(inputs[f"b{i}"], np.float32) for i in range(1, 6)}
    return make_in_maps(
        np.asarray(inputs["x"], np.float32),
        np.asarray(inputs["low"], np.float32),
        np.asarray(inputs["high"], np.float32),
        Ws, bs,
    )


def kernel(**inputs):
    from concourse import bass_utils

    if "nc" not in _CACHE:
        _CACHE["nc"] = build_nc()
    nc = _CACHE["nc"]

    in_maps = _prep_inputs(inputs)
    res = bass_utils.run_bass_kernel_spmd(nc, in_maps,
                                          core_ids=list(range(N_CORES)))
    out = res.results[0]["out"]
    return out[0].copy(), out[1].copy(), out[2].copy()


if __name__ == "__main__":
    import reference

    inp = reference.setup_inputs()
    inp_np = {k: np.asarray(v) for k, v in inp.items()}
    got = kernel(**inp_np)
    exp = reference.reference(**inp)
    for name, g, e in zip(("x", "low", "high"), got, exp):
        e = np.asarray(e)
        err = np.abs(g - e).max() / max(np.abs(e).max(), 1e-9)
        print(f"{name}: rel_err={err:.3e}")
        print("  got:", g[:5])
        print("  exp:", e[:5])

